# revision 1
# baseline (speedup 1.0000x reference)
"""Corr1d cost-volume kernel for Trainium2 (8 NeuronCores).

corr[b, d, h, x] = sum_c fL[b,c,h,x] * fR[b,c,h,x-d]  for x >= d, else 0.
Shapes: fL, fR = (4, 64, 256, 512) fp32; out = (4, 48, 256, 512) fp32.

Sharding: data-parallel over (batch, h-half): core i handles b = i//2,
h rows [128*(i%2), 128*(i%2)+128).

Per-core pipeline (per h row):
  - fp16 cast-load of fL/fR h-batches (SWDGE)
  - 4 banded matmuls (contract c=64 on partitions): lhsT = fL[c, x-block(128)],
    rhs = fR[c, window(176)] -> PSUM [128, 176] fp32
  - DVE copy PSUM -> SBUF fp16 data tile [128, 704]
  - gpsimd local_scatter with a constant per-partition index table: shears the
    diagonal band into a rect [128 x, 192 = 4 blocks x 48 d] (zero-filled)
  - 2 PE transposes [128, 96] -> PSUM [96, 128]
  - ACT copies -> fp32 assembly [96, NH*256]
  - 4 output DMAs per h-batch
"""
import numpy as np
from contextlib import ExitStack

import concourse.bass as bass
import concourse.tile as tile
import concourse.bacc as bacc
import concourse.mybir as mybir
from concourse import bass_utils
from concourse.ap import AP

B, C, H, W = 4, 64, 256, 512
D = 48
NCORES = 8
HH = H // 2            # h rows per core
NH = 8                 # h rows per batch
NBATCH = HH // NH      # 16
WRHS = 176             # rhs window width
W0S = [0, 81, 209, 336]  # rhs window starts per x-block
NBLK = 4

fp16 = mybir.dt.float16
fp32 = mybir.dt.float32
i16 = mybir.dt.int16


def _make_tables():
    idx = np.full((128, NBLK * WRHS), -1, dtype=np.int16)
    for m in range(NBLK):
        base = 128 * m - W0S[m]  # d = base + p - n
        for p in range(128):
            lo = max(0, base + p - (D - 1))
            hi = min(WRHS - 1, base + p)
            for n in range(lo, hi + 1):
                d = base + p - n
                idx[p, WRHS * m + n] = D * m + d
    ident = np.eye(128, dtype=np.float16)
    return idx, ident


def _build_nc():
    nc = bacc.Bacc("TRN2", target_bir_lowering=False, debug=False,
                   num_devices=NCORES)
    fL_d = nc.dram_tensor("fLc", [C, HH, W], fp32, kind="ExternalInput").ap()
    fR_d = nc.dram_tensor("fRc", [C, HH, W], fp32, kind="ExternalInput").ap()
    idx_d = nc.dram_tensor("idx", [128, NBLK * WRHS], i16, kind="ExternalInput").ap()
    ident_d = nc.dram_tensor("ident", [128, 128], fp16, kind="ExternalInput").ap()
    out_d = nc.dram_tensor("outc", [D, HH, W], fp32, kind="ExternalOutput").ap()

    with tile.TileContext(nc) as tc, ExitStack() as ctx:
        const_pool = ctx.enter_context(tc.tile_pool(name="const", bufs=1))
        in_pool = ctx.enter_context(tc.tile_pool(name="inp", bufs=2))
        data_pool = ctx.enter_context(tc.tile_pool(name="data", bufs=3))
        band_pool = ctx.enter_context(tc.tile_pool(name="band", bufs=3))
        asm_pool = ctx.enter_context(tc.tile_pool(name="asm", bufs=2))
        mm_psum = ctx.enter_context(tc.tile_pool(name="mmps", bufs=4, space="PSUM"))
        tp_psum = ctx.enter_context(tc.tile_pool(name="tpps", bufs=3, space="PSUM"))

        idx_t = const_pool.tile([128, NBLK * WRHS], i16)
        nc.sync.dma_start(idx_t[:], idx_d)
        ident_t = const_pool.tile([128, 128], fp16)
        nc.sync.dma_start(ident_t[:], ident_d)

        for ib in range(NBATCH):
            h0 = ib * NH
            fl = in_pool.tile([C, NH * W], fp16, tag="fl")
            nc.gpsimd.dma_start(
                fl[:].rearrange("c (h x) -> c h x", h=NH),
                fL_d[:, h0 : h0 + NH, :],
            )
            fr = in_pool.tile([C, NH * W], fp16, tag="fr")
            nc.gpsimd.dma_start(
                fr[:].rearrange("c (h x) -> c h x", h=NH),
                fR_d[:, h0 : h0 + NH, :],
            )

            asm = asm_pool.tile([96, NH * 256], fp32)

            for hh in range(NH):
                data = data_pool.tile([128, NBLK * WRHS], fp16)
                for m in range(NBLK):
                    ps = mm_psum.tile([128, WRHS], fp32)
                    nc.tensor.matmul(
                        ps[:],
                        fl[:, hh * W + 128 * m : hh * W + 128 * m + 128],
                        fr[:, hh * W + W0S[m] : hh * W + W0S[m] + WRHS],
                        start=True,
                        stop=True,
                    )
                    nc.vector.tensor_copy(
                        data[:, WRHS * m : WRHS * (m + 1)], ps[:]
                    )
                band = band_pool.tile([128, NBLK * D], fp16)
                nc.gpsimd.local_scatter(
                    band[:], data[:], idx_t[:],
                    channels=128, num_elems=NBLK * D, num_idxs=NBLK * WRHS,
                )
                for t in range(2):
                    tp = tp_psum.tile([96, 128], fp16)
                    nc.tensor.transpose(
                        tp[:], band[:, 96 * t : 96 * t + 96], ident_t[:]
                    )
                    nc.scalar.copy(
                        asm[:, hh * 256 + 128 * t : hh * 256 + 128 * t + 128],
                        tp[:],
                    )

            # output DMAs: asm[48*par + d, h*256 + t*128 + xin] ->
            #   out[48m+... d, h0+h, 128*(2t+par)... x = 256t + 128par + xin
            for par in range(2):
                for t in range(2):
                    nc.sync.dma_start(
                        out_d[:, h0 : h0 + NH, 256 * t + 128 * par :
                              256 * t + 128 * par + 128],
                        asm[48 * par : 48 * par + 48, :]
                        .rearrange("d (h x) -> d h x", h=NH)[:, :, 128 * t : 128 * t + 128],
                    )

    nc.compile()
    return nc


_NC_CACHE = None


def _get_nc():
    global _NC_CACHE
    if _NC_CACHE is None:
        _NC_CACHE = _build_nc()
    return _NC_CACHE


def kernel(fL: np.ndarray, fR: np.ndarray) -> np.ndarray:
    fL = np.asarray(fL, dtype=np.float32)
    fR = np.asarray(fR, dtype=np.float32)
    nc = _get_nc()
    idx, ident = _make_tables()

    in_maps = []
    for core in range(NCORES):
        b, half = divmod(core, 2)
        sl = np.s_[b, :, half * HH : half * HH + HH, :]
        in_maps.append({
            "fLc": np.ascontiguousarray(fL[sl]),
            "fRc": np.ascontiguousarray(fR[sl]),
            "idx": idx,
            "ident": ident,
        })

    res = bass_utils.run_bass_kernel_spmd(nc, in_maps, core_ids=list(range(NCORES)))
    out = np.empty((B, D, H, W), dtype=np.float32)
    for core in range(NCORES):
        b, half = divmod(core, 2)
        out[b, :, half * HH : half * HH + HH, :] = res.results[core]["outc"]
    return out
